# revision 11
# baseline (speedup 1.0000x reference)
"""MessagePassingConvolution kernel for 8 Trainium2 NeuronCores.

Strategy:
  - Host: sort edges by receiver; shard by receiver windows. Core m owns
    nodes [m*1280, (m+1)*1280) = 10 windows of 128 nodes. Each window's
    edge list is padded to a fixed budget (2176 = 17 subtiles of 128) so
    the SPMD program is identical across cores. The sender gather
    (node_feats[senders], replicated 4x along partitions) and the
    edge_attrs channel-expansion are done host-side so the device sees
    only sequential streams.
  - Device (per core, per 512/256-edge tile):
      MLP (feature-major matmuls + Silu) -> h3p [64, T] bf16, with W3
        columns permuted so partition k' = 16*j + c holds k = 4c + j.
      h3bc[(j,i), c, e] = h3p[16j + c, e] via DRAM-bounce broadcast
        DMAs split across the sync and gpsimd queues.
      A_c = h3bc_c * Xrep (DVE bf16 2x), u[96,T] += Wg_c.T @ A_c
        (16 matmuls, bf16)
      transpose u -> edge-major ut (bf16), msgs = ut * at_exp (bf16 DVE),
      scatter: psum_out[128n, 288] += S.T @ msgs with S (bf16) built
        on-device from recv_local via iota==scalar compare.
  - Output: per-core [1280, 288] slices -> concat -> [10000, 32, 9].
"""

import sys
import numpy as np
from contextlib import ExitStack

sys.path.insert(0, "/opt/trn_rl_repo")

import concourse.bass as bass  # noqa: E402
import concourse.bacc as bacc  # noqa: E402
import concourse.mybir as mybir  # noqa: E402
import concourse.tile as tile  # noqa: E402
from concourse.masks import make_identity  # noqa: E402
from concourse.bass_utils import run_bass_kernel_spmd  # noqa: E402

import ml_dtypes  # noqa: E402

BF16 = ml_dtypes.bfloat16

# ---- problem constants (hardcoded per spec) ----
N_NODES = 10000
N_EDGES = 160000
C = 32
RADIAL = 8
HID = 64
NL = 3
L_DIMS = (1, 3, 5)
NSH = 9  # sum(L_DIMS)
AVG_NUM_NEIGHBORS = 16.0

N_CORES = 8
WIN = 128                      # nodes per window (psum partitions)
WINS_PER_CORE = 10
NODES_PER_CORE = WIN * WINS_PER_CORE     # 1280
N_NODES_PAD = NODES_PER_CORE * N_CORES   # 10240
SUB = 128                      # edges per subtile
SUBS_PER_WIN = 17              # window edge budget = 2176 (data max 2155)
WIN_E = SUB * SUBS_PER_WIN     # 2176
E_CORE = WIN_E * WINS_PER_CORE  # 21760
N_ST = E_CORE // SUB           # 170 subtiles per core
TILE_SIZES = (512, 512, 512, 512, 128)   # per-window einsum tiles
N_CHUNK = 16                   # ki chunks (2048 / 128)
LO = NL * C                    # 96
F_OUT = NSH * C                # 288

FP32 = mybir.dt.float32
BF16_DT = mybir.dt.bfloat16

_CACHED = {}

# CoreSim doesn't implement Silu; sim_test.py overrides this to Sigmoid and
# checks against a sigmoid-variant reference to validate the data plumbing.
ACT_FUNC = mybir.ActivationFunctionType.Silu


def _build_nc():
    nc = bacc.Bacc()

    ef = nc.dram_tensor("ef", [RADIAL, E_CORE], FP32, kind="ExternalInput")
    x4 = nc.dram_tensor("x4", [128, E_CORE], BF16_DT, kind="ExternalInput")
    atx = nc.dram_tensor("atx", [SUB, N_ST * F_OUT], BF16_DT,
                         kind="ExternalInput")
    rl = nc.dram_tensor("rl", [SUB, N_ST], FP32, kind="ExternalInput")
    w1 = nc.dram_tensor("w1", [RADIAL, HID], FP32, kind="ExternalInput")
    w2 = nc.dram_tensor("w2", [HID, HID], FP32, kind="ExternalInput")
    w3 = nc.dram_tensor("w3", [HID, HID], FP32, kind="ExternalInput")
    wg = nc.dram_tensor("wg", [128, N_CHUNK * LO], BF16_DT, kind="ExternalInput")
    iota = nc.dram_tensor("iota", [128, 128], FP32, kind="ExternalInput")
    out = nc.dram_tensor("out", [NODES_PER_CORE, F_OUT], FP32, kind="ExternalOutput")

    with tile.TileContext(nc) as tc, ExitStack() as ctx:
        const_p = ctx.enter_context(tc.tile_pool(name="const", bufs=1))
        stream_p = ctx.enter_context(tc.tile_pool(name="stream", bufs=3))
        win_p = ctx.enter_context(tc.tile_pool(name="win", bufs=2))
        chunk_p = ctx.enter_context(tc.tile_pool(name="chunk", bufs=3))
        psum_mlp = ctx.enter_context(tc.tile_pool(name="pmlp", bufs=2, space="PSUM"))
        psum_u = ctx.enter_context(tc.tile_pool(name="pu", bufs=3, space="PSUM"))
        psum_ut = ctx.enter_context(tc.tile_pool(name="put", bufs=2, space="PSUM"))
        psum_acc = ctx.enter_context(tc.tile_pool(name="pacc", bufs=1, space="PSUM"))
        dram_p = ctx.enter_context(tc.tile_pool(name="dram", bufs=3, space="DRAM"))

        # ---- one-time constants into SBUF ----
        w1_sb = const_p.tile([RADIAL, HID], FP32)
        nc.scalar.dma_start(w1_sb[:], w1[:])
        w2_sb = const_p.tile([HID, HID], FP32)
        nc.scalar.dma_start(w2_sb[:], w2[:])
        w3_sb = const_p.tile([HID, HID], FP32)
        nc.scalar.dma_start(w3_sb[:], w3[:])
        wg_sb = const_p.tile([128, N_CHUNK * LO], BF16_DT)
        nc.scalar.dma_start(wg_sb[:], wg[:])
        iota_sb = const_p.tile([128, 128], FP32)
        nc.scalar.dma_start(iota_sb[:], iota[:])
        ident_sb = const_p.tile([128, 128], FP32)
        make_identity(nc, ident_sb[:])

        for w in range(WINS_PER_CORE):
            # window-level streams
            atx_sb = win_p.tile([SUB, SUBS_PER_WIN * F_OUT], BF16_DT, tag="at")
            nc.scalar.dma_start(
                atx_sb[:],
                atx[:, w * SUBS_PER_WIN * F_OUT:(w + 1) * SUBS_PER_WIN * F_OUT])
            rl_sb = win_p.tile([SUB, SUBS_PER_WIN], FP32, tag="rl")
            nc.scalar.dma_start(
                rl_sb[:], rl[:, w * SUBS_PER_WIN:(w + 1) * SUBS_PER_WIN])
            ut_sb = win_p.tile([SUB, SUBS_PER_WIN, LO], BF16_DT, tag="ut")
            msgs_sb = win_p.tile([SUB, SUBS_PER_WIN, F_OUT], BF16_DT, tag="msgs")

            e_off = 0  # edge offset within window
            for tsz in TILE_SIZES:
                base = w * WIN_E + e_off          # global edge-slot offset
                nsub = tsz // SUB

                ef_sb = stream_p.tile([RADIAL, 512], FP32, tag="ef")
                nc.scalar.dma_start(ef_sb[:, :tsz], ef[:, base:base + tsz])

                # Xrep[p, e] = node_feats[senders[e], p % 32] (host-gathered)
                x_sb = stream_p.tile([128, 512], BF16_DT, tag="x")
                nc.scalar.dma_start(x_sb[:, :tsz], x4[:, base:base + tsz])

                # --- MLP (feature-major) ---
                z1 = psum_mlp.tile([HID, 512], FP32, tag="z")
                nc.tensor.matmul(out=z1[:, :tsz], lhsT=w1_sb[:], rhs=ef_sb[:, :tsz],
                                 start=True, stop=True, skip_group_check=True)
                h1 = stream_p.tile([HID, 512], FP32, tag="h1")
                nc.scalar.activation(h1[:, :tsz], z1[:, :tsz],
                                     ACT_FUNC)
                z2 = psum_mlp.tile([HID, 512], FP32, tag="z")
                nc.tensor.matmul(out=z2[:, :tsz], lhsT=w2_sb[:], rhs=h1[:, :tsz],
                                 start=True, stop=True, skip_group_check=True)
                h2 = stream_p.tile([HID, 512], FP32, tag="h2")
                nc.scalar.activation(h2[:, :tsz], z2[:, :tsz],
                                     ACT_FUNC)
                # last layer with W3 columns permuted: row k' = 16j + c is
                # MLP feature k = 4c + j
                z3 = psum_mlp.tile([HID, 512], FP32, tag="z")
                nc.tensor.matmul(out=z3[:, :tsz], lhsT=w3_sb[:], rhs=h2[:, :tsz],
                                 start=True, stop=True, skip_group_check=True)
                h3p = stream_p.tile([HID, 512], BF16_DT, tag="h3")
                nc.scalar.activation(h3p[:, :tsz], z3[:, :tsz],
                                     ACT_FUNC)

                # --- broadcast h3 along partitions (32x) via DRAM bounce ---
                # h3p rows are already (j, c)-ordered, so the bounce write is
                # a plain copy; reads are split across sync + gpsimd queues.
                # h3bc[p=(j,i), c, e] = h3p[16j + c, e]
                h3d = dram_p.tile([HID, 512], BF16_DT, tag="h3d")
                nc.sync.dma_start(h3d[:, :tsz], h3p[:, :tsz])
                h3bc = chunk_p.tile([128, N_CHUNK, 512], BF16_DT, tag="h3bc")
                if tsz == 512:
                    for jp, eng in ((0, nc.sync), (2, nc.scalar)):
                        src = h3d[16 * jp:16 * jp + 32, :].rearrange(
                            "(j c) e -> j (c e)", j=2)
                        src = src[:, None, :].to_broadcast([2, 32, N_CHUNK * 512])
                        eng.dma_start(
                            h3bc[32 * jp:32 * jp + 64].rearrange(
                                "p c e -> p (c e)"), src)
                else:
                    for j in range(4):
                        src = h3d[16 * j:16 * (j + 1), :tsz]
                        src = src[None, :, :].to_broadcast([32, N_CHUNK, tsz])
                        eng = nc.sync if j < 2 else nc.scalar
                        eng.dma_start(h3bc[32 * j:32 * (j + 1), :, :tsz], src)

                # --- outer product (batched) + einsum chunks ---
                a_all = chunk_p.tile([128, N_CHUNK, 512], BF16_DT, tag="a")
                for g in range(4):
                    nc.vector.tensor_tensor(
                        out=a_all[:, 4 * g:4 * g + 4, :tsz],
                        in0=h3bc[:, 4 * g:4 * g + 4, :tsz],
                        in1=x_sb[:, None, :tsz].to_broadcast([128, 4, tsz]),
                        op=mybir.AluOpType.mult)
                u_ps = psum_u.tile([LO, 512], FP32, tag="u")
                for c in range(N_CHUNK):
                    nc.tensor.matmul(out=u_ps[:, :tsz],
                                     lhsT=wg_sb[:, c * LO:(c + 1) * LO],
                                     rhs=a_all[:, c, :tsz],
                                     start=(c == 0), stop=(c == N_CHUNK - 1),
                                     skip_group_check=True)

                # --- transpose u to edge-major ---
                u_sb = stream_p.tile([LO, 512], FP32, tag="usb")
                nc.scalar.copy(u_sb[:, :tsz], u_ps[:, :tsz])
                ut_ps = psum_ut.tile([128, 4, LO], FP32, tag="utp")
                for s in range(nsub):
                    nc.tensor.transpose(
                        out=ut_ps[:, s, :],
                        in_=u_sb[:, s * SUB:(s + 1) * SUB],
                        identity=ident_sb[:LO, :LO])
                st0 = e_off // SUB
                nc.scalar.copy(ut_sb[:, st0:st0 + nsub, :], ut_ps[:, :nsub, :])

                e_off += tsz

            # --- msgs = ut * at_exp (both bf16, fully contiguous innermost) ---
            # at_exp already has each attr value repeated C times along cols.
            lofs = (0, 1, 4)
            for l in range(NL):
                dim = L_DIMS[l]
                u_ap = ut_sb[:, :, None, l * C:(l + 1) * C].to_broadcast(
                    [SUB, SUBS_PER_WIN, dim, C])
                a_ap = atx_sb[:].rearrange(
                    "p (s m c) -> p s m c", m=NSH, c=C)[
                        :, :, lofs[l]:lofs[l] + dim, :]
                nc.vector.tensor_tensor(
                    out=msgs_sb[:, :, lofs[l] * C:(lofs[l] + dim) * C].rearrange(
                        "p s (m c) -> p s m c", c=C),
                    in0=u_ap, in1=a_ap, op=mybir.AluOpType.mult)

            # --- scatter: psum_out += S.T @ msgs per subtile ---
            # S for all subtiles built in one DVE op (overhead-bound otherwise)
            s_all = chunk_p.tile([SUB, SUBS_PER_WIN, WIN], BF16_DT, tag="s")
            nc.vector.tensor_tensor(
                out=s_all[:],
                in0=iota_sb[:, None, :].to_broadcast([SUB, SUBS_PER_WIN, WIN]),
                in1=rl_sb[:, :, None].to_broadcast([SUB, SUBS_PER_WIN, WIN]),
                op=mybir.AluOpType.is_equal)
            acc = psum_acc.tile([WIN, F_OUT], FP32, tag="acc")
            for st in range(SUBS_PER_WIN):
                nc.tensor.matmul(out=acc[:], lhsT=s_all[:, st, :],
                                 rhs=msgs_sb[:, st, :],
                                 start=(st == 0), stop=(st == SUBS_PER_WIN - 1),
                                 skip_group_check=True)

            out_sb = stream_p.tile([WIN, F_OUT], FP32, tag="osb")
            nc.scalar.copy(out_sb[:], acc[:])
            nc.scalar.dma_start(out[w * WIN:(w + 1) * WIN, :], out_sb[:])

    nc.compile()
    return nc


def _host_prep(node_feats, edge_attrs, edge_feats, senders, receivers,
               W1, W2, W3, Wgen):
    """Sort/shard edges by receiver window, build per-core input maps."""
    senders = np.asarray(senders).astype(np.int64)
    receivers = np.asarray(receivers).astype(np.int64)
    node_feats = np.asarray(node_feats, dtype=np.float32)
    edge_attrs = np.asarray(edge_attrs, dtype=np.float32)
    edge_feats = np.asarray(edge_feats, dtype=np.float32)

    n_win_total = N_CORES * WINS_PER_CORE  # 80
    win_id = receivers // WIN
    order = np.argsort(win_id, kind="stable")
    counts = np.bincount(win_id, minlength=n_win_total)
    assert counts.max() <= WIN_E, f"window overflow: {counts.max()} > {WIN_E}"
    starts = np.zeros(n_win_total + 1, np.int64)
    np.cumsum(counts, out=starts[1:])

    # slot arrays (padded); padding edges: ef=0, attr=0 -> msgs contribution 0
    E_TOT = N_CORES * E_CORE
    ef_s = np.zeros((E_TOT, RADIAL), np.float32)
    at_s = np.zeros((E_TOT, NSH), np.float32)
    rl_s = np.zeros(E_TOT, np.float32)
    sd_s = np.zeros(E_TOT, np.int64)

    slot_base = np.arange(n_win_total) * WIN_E
    # positions for real edges
    within = np.arange(len(order)) - starts[win_id[order]]
    slots = slot_base[win_id[order]] + within
    ef_s[slots] = edge_feats[order]
    at_s[slots] = edge_attrs[order] * np.float32(1.0 / np.sqrt(AVG_NUM_NEIGHBORS))
    rl_s[slots] = (receivers[order] % WIN).astype(np.float32)
    sd_s[slots] = senders[order]

    # host-side sender gather, replicated 4x along partitions (bf16)
    xg = node_feats[sd_s].astype(BF16)            # [E_TOT, 32]

    # weights with fan-in scales folded
    w1 = (W1 * (1.0 / np.sqrt(RADIAL))).astype(np.float32)
    w2 = (W2 * (1.0 / np.sqrt(HID))).astype(np.float32)
    w3 = (W3 * (1.0 / np.sqrt(HID))).astype(np.float32)
    # permute W3 columns so output row k' = 16j + c holds feature k = 4c + j
    perm = np.empty(HID, np.int64)
    for j in range(4):
        for c in range(N_CHUNK):
            perm[16 * j + c] = 4 * c + j
    w3p = np.ascontiguousarray(w3[:, perm])
    # wg[c*128+p, l*32+o] = Wgen[4c + p//32, l, o, p%32] * 1/sqrt(HID*C)
    wgen = np.asarray(Wgen, dtype=np.float32) * np.float32(1.0 / np.sqrt(HID * C))
    p = np.arange(128)
    wgc = np.zeros((N_CHUNK, 128, NL, C), np.float32)
    for c in range(N_CHUNK):
        wgc[c] = wgen[4 * c + p // 32][p, :, :, p % 32].reshape(128, NL, C)
    # -> [128, 16*96]: chunk-major along free dim
    wgc = wgc.reshape(N_CHUNK, 128, LO).transpose(1, 0, 2).reshape(
        128, N_CHUNK * LO)
    wgc = wgc.astype(BF16)

    iota = np.broadcast_to(np.arange(128, dtype=np.float32),
                           (128, 128)).copy()

    in_maps = []
    for m in range(N_CORES):
        sl = slice(m * E_CORE, (m + 1) * E_CORE)
        ef_c = ef_s[sl]      # [E_CORE, 8]
        at_c = at_s[sl]      # [E_CORE, 9]
        rl_c = rl_s[sl]
        # attrs expanded: col m*C + c = attr[m], bf16, subtile-major
        atx_c = np.repeat(at_c, C, axis=1).astype(BF16)       # [E_CORE, 288]
        atx_c = np.ascontiguousarray(
            atx_c.reshape(N_ST, SUB, F_OUT).transpose(1, 0, 2).reshape(
                SUB, N_ST * F_OUT))
        x4_c = np.ascontiguousarray(
            np.tile(xg[sl].T, (4, 1)))                        # [128, E_CORE]
        in_maps.append({
            "ef": np.ascontiguousarray(ef_c.T),
            "atx": atx_c,
            "rl": np.ascontiguousarray(
                rl_c.reshape(N_ST, SUB).T),
            "x4": x4_c,
            "w1": w1, "w2": w2, "w3": w3p, "wg": wgc,
            "iota": iota,
        })
    return in_maps


def kernel(node_feats, edge_attrs, edge_feats, senders, receivers,
           W1, W2, W3, Wgen):
    in_maps = _host_prep(node_feats, edge_attrs, edge_feats, senders, receivers,
                         W1, W2, W3, Wgen)
    if "nc" not in _CACHED:
        _CACHED["nc"] = _build_nc()
    nc = _CACHED["nc"]
    res = run_bass_kernel_spmd(nc, in_maps, core_ids=list(range(N_CORES)))
    outs = [res.results[m]["out"] for m in range(N_CORES)]
    full = np.concatenate(outs, axis=0)[:N_NODES]          # [10000, 288]
    out = full.reshape(N_NODES, NSH, C).transpose(0, 2, 1)  # [10000, 32, 9]
    return np.ascontiguousarray(out.astype(np.float32))


# revision 18
# speedup vs baseline: 4.6346x; 4.6346x over previous
"""MessagePassingConvolution kernel for 8 Trainium2 NeuronCores.

Strategy:
  - Host: sort edges by receiver; shard by receiver windows. Core m owns
    nodes [m*1280, (m+1)*1280) = 10 windows of 128 nodes. Each window's
    edge list is padded to a fixed budget (2176 = 17 subtiles of 128) so
    the SPMD program is identical across cores. The sender gather
    (node_feats[senders], replicated 4x along partitions) and the
    edge_attrs channel-expansion are done host-side so the device sees
    only sequential streams.
  - Device (per core, per 512/256-edge tile):
      MLP (feature-major matmuls + Silu) -> h3p [64, T] bf16, with W3
        columns permuted so partition k' = 16*j + c holds k = 4c + j.
      h3bc[(j,i), c, e] = h3p[16j + c, e] via DRAM-bounce broadcast
        DMAs split across the sync and gpsimd queues.
      A_c = h3bc_c * Xrep (DVE bf16 2x), u[96,T] += Wg_c.T @ A_c
        (16 matmuls, bf16)
      transpose u -> edge-major ut (bf16), msgs = ut * at_exp (bf16 DVE),
      scatter: psum_out[128n, 288] += S.T @ msgs with S (bf16) built
        on-device from recv_local via iota==scalar compare.
  - Output: per-core [1280, 288] slices -> concat -> [10000, 32, 9].
"""

import sys
import numpy as np
from contextlib import ExitStack

sys.path.insert(0, "/opt/trn_rl_repo")

import concourse.bass as bass  # noqa: E402
import concourse.bacc as bacc  # noqa: E402
import concourse.mybir as mybir  # noqa: E402
import concourse.tile as tile  # noqa: E402
from concourse.masks import make_identity  # noqa: E402
from concourse.bass_utils import run_bass_kernel_spmd  # noqa: E402

import ml_dtypes  # noqa: E402

BF16 = ml_dtypes.bfloat16

# ---- problem constants (hardcoded per spec) ----
N_NODES = 10000
N_EDGES = 160000
C = 32
RADIAL = 8
HID = 64
NL = 3
L_DIMS = (1, 3, 5)
NSH = 9  # sum(L_DIMS)
AVG_NUM_NEIGHBORS = 16.0

N_CORES = 8
WIN = 128                      # nodes per window (psum partitions)
WINS_PER_CORE = 10
NODES_PER_CORE = WIN * WINS_PER_CORE     # 1280
N_NODES_PAD = NODES_PER_CORE * N_CORES   # 10240
SUB = 128                      # edges per subtile
SUBS_PER_WIN = 17              # window edge budget = 2176 (data max 2155)
WIN_E = SUB * SUBS_PER_WIN     # 2176
E_CORE = WIN_E * WINS_PER_CORE  # 21760
N_ST = E_CORE // SUB           # 170 subtiles per core
TILE_SIZES = (512, 512, 512, 512, 128)   # per-window einsum tiles
N_CHUNK = 16                   # ki chunks (2048 / 128)
LO = NL * C                    # 96
F_OUT = NSH * C                # 288

FP32 = mybir.dt.float32
BF16_DT = mybir.dt.bfloat16

_CACHED = {}

# CoreSim doesn't implement Silu; sim_test.py overrides this to Sigmoid and
# checks against a sigmoid-variant reference to validate the data plumbing.
ACT_FUNC = mybir.ActivationFunctionType.Silu


def _build_nc():
    nc = bacc.Bacc()

    ef = nc.dram_tensor("ef", [RADIAL, E_CORE], FP32, kind="ExternalInput")
    x4 = nc.dram_tensor("x4", [128, E_CORE], BF16_DT, kind="ExternalInput")
    atx = nc.dram_tensor("atx", [SUB, N_ST * F_OUT], BF16_DT,
                         kind="ExternalInput")
    rl = nc.dram_tensor("rl", [SUB, N_ST], FP32, kind="ExternalInput")
    w1 = nc.dram_tensor("w1", [RADIAL, HID], FP32, kind="ExternalInput")
    w2 = nc.dram_tensor("w2", [HID, HID], FP32, kind="ExternalInput")
    w3 = nc.dram_tensor("w3", [HID, HID], FP32, kind="ExternalInput")
    wg = nc.dram_tensor("wg", [128, N_CHUNK * LO], BF16_DT, kind="ExternalInput")
    iota = nc.dram_tensor("iota", [128, 128], FP32, kind="ExternalInput")
    out = nc.dram_tensor("out", [NODES_PER_CORE, F_OUT], FP32, kind="ExternalOutput")

    with tile.TileContext(nc) as tc, ExitStack() as ctx:
        const_p = ctx.enter_context(tc.tile_pool(name="const", bufs=1))
        stream_p = ctx.enter_context(tc.tile_pool(name="stream", bufs=3))
        win_p = ctx.enter_context(tc.tile_pool(name="win", bufs=2))
        chunk_p = ctx.enter_context(tc.tile_pool(name="chunk", bufs=3))
        psum_mlp = ctx.enter_context(tc.tile_pool(name="pmlp", bufs=2, space="PSUM"))
        psum_u = ctx.enter_context(tc.tile_pool(name="pu", bufs=3, space="PSUM"))
        psum_ut = ctx.enter_context(tc.tile_pool(name="put", bufs=2, space="PSUM"))
        psum_acc = ctx.enter_context(tc.tile_pool(name="pacc", bufs=1, space="PSUM"))
        dram_p = ctx.enter_context(tc.tile_pool(name="dram", bufs=3, space="DRAM"))
        tail_p = ctx.enter_context(tc.tile_pool(name="tail", bufs=1))

        # ---- one-time constants into SBUF ----
        w1_sb = const_p.tile([RADIAL, HID], FP32)
        nc.scalar.dma_start(w1_sb[:], w1[:])
        w2_sb = const_p.tile([HID, HID], FP32)
        nc.scalar.dma_start(w2_sb[:], w2[:])
        w3_sb = const_p.tile([HID, HID], FP32)
        nc.scalar.dma_start(w3_sb[:], w3[:])
        wg_sb = const_p.tile([128, N_CHUNK * LO], BF16_DT)
        nc.scalar.dma_start(wg_sb[:], wg[:])
        iota_sb = const_p.tile([128, 128], FP32)
        nc.scalar.dma_start(iota_sb[:], iota[:])
        ident_sb = const_p.tile([128, 128], FP32)
        make_identity(nc, ident_sb[:])

        for w in range(WINS_PER_CORE):
            # window-level streams
            atx_sb = win_p.tile([SUB, SUBS_PER_WIN * F_OUT], BF16_DT, tag="at")
            nc.scalar.dma_start(
                atx_sb[:],
                atx[:, w * SUBS_PER_WIN * F_OUT:(w + 1) * SUBS_PER_WIN * F_OUT])
            rl_sb = win_p.tile([SUB, SUBS_PER_WIN], FP32, tag="rl")
            nc.scalar.dma_start(
                rl_sb[:], rl[:, w * SUBS_PER_WIN:(w + 1) * SUBS_PER_WIN])
            ut_sb = win_p.tile([SUB, SUBS_PER_WIN, LO], BF16_DT, tag="ut")
            msgs_sb = win_p.tile([SUB, SUBS_PER_WIN, F_OUT], BF16_DT, tag="msgs")

            e_off = 0  # edge offset within window
            for tsz in TILE_SIZES:
                base = w * WIN_E + e_off          # global edge-slot offset
                nsub = tsz // SUB

                ef_sb = stream_p.tile([RADIAL, 512], FP32, tag="ef")
                nc.scalar.dma_start(ef_sb[:, :tsz], ef[:, base:base + tsz])

                # Xrep[p, e] = node_feats[senders[e], p // 4] (host-gathered)
                x_sb = stream_p.tile([128, 512], BF16_DT, tag="x")
                nc.scalar.dma_start(x_sb[:, :tsz], x4[:, base:base + tsz])

                # --- MLP (feature-major) ---
                z1 = psum_mlp.tile([HID, 512], FP32, tag="z")
                nc.tensor.matmul(out=z1[:, :tsz], lhsT=w1_sb[:], rhs=ef_sb[:, :tsz],
                                 start=True, stop=True, skip_group_check=True)
                h1 = stream_p.tile([HID, 512], FP32, tag="h1")
                nc.scalar.activation(h1[:, :tsz], z1[:, :tsz],
                                     ACT_FUNC)
                z2 = psum_mlp.tile([HID, 512], FP32, tag="z")
                nc.tensor.matmul(out=z2[:, :tsz], lhsT=w2_sb[:], rhs=h1[:, :tsz],
                                 start=True, stop=True, skip_group_check=True)
                h2 = stream_p.tile([HID, 512], FP32, tag="h2")
                nc.scalar.activation(h2[:, :tsz], z2[:, :tsz],
                                     ACT_FUNC)
                # last layer with W3 columns permuted: row k' = 16j + c is
                # MLP feature k = 4c + j
                z3 = psum_mlp.tile([HID, 512], FP32, tag="z")
                nc.tensor.matmul(out=z3[:, :tsz], lhsT=w3_sb[:], rhs=h2[:, :tsz],
                                 start=True, stop=True, skip_group_check=True)
                h3p = stream_p.tile([HID, 512], BF16_DT, tag="h3")
                nc.scalar.activation(h3p[:, :tsz], z3[:, :tsz],
                                     ACT_FUNC)

                # --- broadcast h3 along partitions (32x) via DRAM bounce ---
                # h3p rows are already (j, c)-ordered, so the bounce write is
                # a plain copy; reads are split across sync + gpsimd queues.
                # h3bc[p=(j,i), c, e] = h3p[16j + c, e]
                # The DGE splits a DMA across SDMA engines by the OUTERMOST
                # AP dim, so put the 32-way replication dim outermost to
                # engage all 16 engines (outer dim 4 -> only 4 engines).
                # Partition convention p = 4i + j: the dest partition dim
                # pairs positionally with src dims (32, 4, c*e), making the
                # 32-way broadcast the OUTER split factor so the DGE spreads
                # descriptors over 16 SDMA engines (outer dim 4 -> only 4).
                pool = chunk_p if tsz == 512 else tail_p
                h3d = dram_p.tile([HID, tsz], BF16_DT, tag=f"h3d{tsz}")
                nc.sync.dma_start(h3d[:], h3p[:, :tsz])
                h3bc = pool.tile([128, N_CHUNK, tsz], BF16_DT, tag=f"h3bc{tsz}")
                dst = h3bc[:].rearrange("p c e -> p (c e)")
                src = h3d[:].rearrange("(j c) e -> j (c e)", j=4)
                src = src[None, :, :].to_broadcast([32, 4, N_CHUNK * tsz])
                nc.sync.dma_start(dst, src)

                # --- outer product (batched) + einsum chunks ---
                a_all = pool.tile([128, N_CHUNK, tsz], BF16_DT, tag=f"a{tsz}")
                for g in range(4):
                    nc.vector.tensor_tensor(
                        out=a_all[:, 4 * g:4 * g + 4, :tsz],
                        in0=h3bc[:, 4 * g:4 * g + 4, :tsz],
                        in1=x_sb[:, None, :tsz].to_broadcast([128, 4, tsz]),
                        op=mybir.AluOpType.mult)
                u_ps = psum_u.tile([LO, 512], FP32, tag="u")
                for c in range(N_CHUNK):
                    nc.tensor.matmul(out=u_ps[:, :tsz],
                                     lhsT=wg_sb[:, c * LO:(c + 1) * LO],
                                     rhs=a_all[:, c, :tsz],
                                     start=(c == 0), stop=(c == N_CHUNK - 1),
                                     skip_group_check=True)

                # --- transpose u to edge-major ---
                u_sb = stream_p.tile([LO, 512], FP32, tag="usb")
                nc.scalar.copy(u_sb[:, :tsz], u_ps[:, :tsz])
                ut_ps = psum_ut.tile([128, 4, LO], FP32, tag="utp")
                for s in range(nsub):
                    nc.tensor.transpose(
                        out=ut_ps[:, s, :],
                        in_=u_sb[:, s * SUB:(s + 1) * SUB],
                        identity=ident_sb[:LO, :LO])
                st0 = e_off // SUB
                nc.scalar.copy(ut_sb[:, st0:st0 + nsub, :], ut_ps[:, :nsub, :])

                e_off += tsz

            # --- msgs = ut * at_exp (both bf16, fully contiguous innermost) ---
            # at_exp already has each attr value repeated C times along cols.
            lofs = (0, 1, 4)
            for l in range(NL):
                dim = L_DIMS[l]
                u_ap = ut_sb[:, :, None, l * C:(l + 1) * C].to_broadcast(
                    [SUB, SUBS_PER_WIN, dim, C])
                a_ap = atx_sb[:].rearrange(
                    "p (s m c) -> p s m c", m=NSH, c=C)[
                        :, :, lofs[l]:lofs[l] + dim, :]
                nc.vector.tensor_tensor(
                    out=msgs_sb[:, :, lofs[l] * C:(lofs[l] + dim) * C].rearrange(
                        "p s (m c) -> p s m c", c=C),
                    in0=u_ap, in1=a_ap, op=mybir.AluOpType.mult)

            # --- scatter: psum_out += S.T @ msgs per subtile ---
            # S for all subtiles built in one DVE op (overhead-bound otherwise)
            s_all = chunk_p.tile([SUB, SUBS_PER_WIN, WIN], BF16_DT, tag="s")
            nc.vector.tensor_tensor(
                out=s_all[:],
                in0=iota_sb[:, None, :].to_broadcast([SUB, SUBS_PER_WIN, WIN]),
                in1=rl_sb[:, :, None].to_broadcast([SUB, SUBS_PER_WIN, WIN]),
                op=mybir.AluOpType.is_equal)
            acc = psum_acc.tile([WIN, F_OUT], FP32, tag="acc")
            for st in range(SUBS_PER_WIN):
                nc.tensor.matmul(out=acc[:], lhsT=s_all[:, st, :],
                                 rhs=msgs_sb[:, st, :],
                                 start=(st == 0), stop=(st == SUBS_PER_WIN - 1),
                                 skip_group_check=True)

            out_sb = stream_p.tile([WIN, F_OUT], FP32, tag="osb")
            nc.scalar.copy(out_sb[:], acc[:])
            nc.scalar.dma_start(out[w * WIN:(w + 1) * WIN, :], out_sb[:])

    nc.compile()
    return nc


def _host_prep(node_feats, edge_attrs, edge_feats, senders, receivers,
               W1, W2, W3, Wgen):
    """Sort/shard edges by receiver window, build per-core input maps."""
    senders = np.asarray(senders).astype(np.int64)
    receivers = np.asarray(receivers).astype(np.int64)
    node_feats = np.asarray(node_feats, dtype=np.float32)
    edge_attrs = np.asarray(edge_attrs, dtype=np.float32)
    edge_feats = np.asarray(edge_feats, dtype=np.float32)

    n_win_total = N_CORES * WINS_PER_CORE  # 80
    win_id = receivers // WIN
    order = np.argsort(win_id, kind="stable")
    counts = np.bincount(win_id, minlength=n_win_total)
    assert counts.max() <= WIN_E, f"window overflow: {counts.max()} > {WIN_E}"
    starts = np.zeros(n_win_total + 1, np.int64)
    np.cumsum(counts, out=starts[1:])

    # slot arrays (padded); padding edges: ef=0, attr=0 -> msgs contribution 0
    E_TOT = N_CORES * E_CORE
    ef_s = np.zeros((E_TOT, RADIAL), np.float32)
    at_s = np.zeros((E_TOT, NSH), np.float32)
    rl_s = np.zeros(E_TOT, np.float32)
    sd_s = np.zeros(E_TOT, np.int64)

    slot_base = np.arange(n_win_total) * WIN_E
    # positions for real edges
    within = np.arange(len(order)) - starts[win_id[order]]
    slots = slot_base[win_id[order]] + within
    ef_s[slots] = edge_feats[order]
    at_s[slots] = edge_attrs[order] * np.float32(1.0 / np.sqrt(AVG_NUM_NEIGHBORS))
    rl_s[slots] = (receivers[order] % WIN).astype(np.float32)
    sd_s[slots] = senders[order]

    # host-side sender gather, replicated 4x along partitions (bf16)
    xg = node_feats[sd_s].astype(BF16)            # [E_TOT, 32]

    # weights with fan-in scales folded
    w1 = (W1 * (1.0 / np.sqrt(RADIAL))).astype(np.float32)
    w2 = (W2 * (1.0 / np.sqrt(HID))).astype(np.float32)
    w3 = (W3 * (1.0 / np.sqrt(HID))).astype(np.float32)
    # permute W3 columns so output row k' = 16j + c holds feature k = 4c + j
    perm = np.empty(HID, np.int64)
    for j in range(4):
        for c in range(N_CHUNK):
            perm[16 * j + c] = 4 * c + j
    w3p = np.ascontiguousarray(w3[:, perm])
    # partition p = 4i + j: wg[c][p, lo] = Wgen[4c + p%4, l, o, p//4] / sqrt(HID*C)
    wgen = np.asarray(Wgen, dtype=np.float32) * np.float32(1.0 / np.sqrt(HID * C))
    p = np.arange(128)
    wgc = np.zeros((N_CHUNK, 128, NL, C), np.float32)
    for c in range(N_CHUNK):
        wgc[c] = wgen[4 * c + p % 4][p, :, :, p // 4].reshape(128, NL, C)
    # -> [128, 16*96]: chunk-major along free dim
    wgc = wgc.reshape(N_CHUNK, 128, LO).transpose(1, 0, 2).reshape(
        128, N_CHUNK * LO)
    wgc = wgc.astype(BF16)

    iota = np.broadcast_to(np.arange(128, dtype=np.float32),
                           (128, 128)).copy()

    in_maps = []
    for m in range(N_CORES):
        sl = slice(m * E_CORE, (m + 1) * E_CORE)
        ef_c = ef_s[sl]      # [E_CORE, 8]
        at_c = at_s[sl]      # [E_CORE, 9]
        rl_c = rl_s[sl]
        # attrs expanded: col m*C + c = attr[m], bf16, subtile-major
        atx_c = np.repeat(at_c, C, axis=1).astype(BF16)       # [E_CORE, 288]
        atx_c = np.ascontiguousarray(
            atx_c.reshape(N_ST, SUB, F_OUT).transpose(1, 0, 2).reshape(
                SUB, N_ST * F_OUT))
        x4_c = np.ascontiguousarray(
            np.repeat(xg[sl].T, 4, axis=0))                   # [128, E_CORE]
        in_maps.append({
            "ef": np.ascontiguousarray(ef_c.T),
            "atx": atx_c,
            "rl": np.ascontiguousarray(
                rl_c.reshape(N_ST, SUB).T),
            "x4": x4_c,
            "w1": w1, "w2": w2, "w3": w3p, "wg": wgc,
            "iota": iota,
        })
    return in_maps


def kernel(node_feats, edge_attrs, edge_feats, senders, receivers,
           W1, W2, W3, Wgen):
    in_maps = _host_prep(node_feats, edge_attrs, edge_feats, senders, receivers,
                         W1, W2, W3, Wgen)
    if "nc" not in _CACHED:
        _CACHED["nc"] = _build_nc()
    nc = _CACHED["nc"]
    res = run_bass_kernel_spmd(nc, in_maps, core_ids=list(range(N_CORES)))
    outs = [res.results[m]["out"] for m in range(N_CORES)]
    full = np.concatenate(outs, axis=0)[:N_NODES]          # [10000, 288]
    out = full.reshape(N_NODES, NSH, C).transpose(0, 2, 1)  # [10000, 32, 9]
    return np.ascontiguousarray(out.astype(np.float32))


# revision 26
# speedup vs baseline: 4.8059x; 1.0370x over previous
"""MessagePassingConvolution kernel for 8 Trainium2 NeuronCores.

Strategy:
  - Host: sort edges by receiver; shard by receiver windows. Core m owns
    nodes [m*1280, (m+1)*1280) = 10 windows of 128 nodes. Each window's
    edge list is padded to a fixed budget (2176 = 17 subtiles of 128) so
    the SPMD program is identical across cores. The sender gather
    (node_feats[senders], replicated 4x along partitions) and the
    edge_attrs channel-expansion are done host-side so the device sees
    only sequential streams.
  - Device (per core, per 512/256-edge tile):
      MLP (feature-major matmuls + Silu) -> h3p [64, T] bf16, with W3
        columns permuted so partition k' = 16*j + c holds k = 4c + j.
      h3bc[(j,i), c, e] = h3p[16j + c, e] via DRAM-bounce broadcast
        DMAs split across the sync and gpsimd queues.
      A_c = h3bc_c * Xrep (DVE bf16 2x), u[96,T] += Wg_c.T @ A_c
        (16 matmuls, bf16)
      transpose u -> edge-major ut (bf16), msgs = ut * at_exp (bf16 DVE),
      scatter: psum_out[128n, 288] += S.T @ msgs with S (bf16) built
        on-device from recv_local via iota==scalar compare.
  - Output: per-core [1280, 288] slices -> concat -> [10000, 32, 9].
"""

import sys
import numpy as np
from contextlib import ExitStack

sys.path.insert(0, "/opt/trn_rl_repo")

import concourse.bass as bass  # noqa: E402
import concourse.bacc as bacc  # noqa: E402
import concourse.mybir as mybir  # noqa: E402
import concourse.tile as tile  # noqa: E402
from concourse.masks import make_identity  # noqa: E402
from concourse.bass_utils import run_bass_kernel_spmd  # noqa: E402

import ml_dtypes  # noqa: E402

BF16 = ml_dtypes.bfloat16

# ---- problem constants (hardcoded per spec) ----
N_NODES = 10000
N_EDGES = 160000
C = 32
RADIAL = 8
HID = 64
NL = 3
L_DIMS = (1, 3, 5)
NSH = 9  # sum(L_DIMS)
AVG_NUM_NEIGHBORS = 16.0

N_CORES = 8
WIN = 128                      # nodes per window (psum partitions)
WINS_PER_CORE = 10
NODES_PER_CORE = WIN * WINS_PER_CORE     # 1280
N_NODES_PAD = NODES_PER_CORE * N_CORES   # 10240
SUB = 128                      # edges per subtile
SUBS_PER_WIN = 17              # window edge budget = 2176 (data max 2155)
WIN_E = SUB * SUBS_PER_WIN     # 2176
E_CORE = WIN_E * WINS_PER_CORE  # 21760
N_ST = E_CORE // SUB           # 170 subtiles per core
TILE_SIZES = (512, 512, 512, 512, 128)   # per-window einsum tiles
N_CHUNK = 16                   # ki chunks (2048 / 128)
LO = NL * C                    # 96
F_OUT = NSH * C                # 288

FP32 = mybir.dt.float32
BF16_DT = mybir.dt.bfloat16

_CACHED = {}

# CoreSim doesn't implement Silu; sim_test.py overrides this to Sigmoid and
# checks against a sigmoid-variant reference to validate the data plumbing.
ACT_FUNC = mybir.ActivationFunctionType.Silu


def _build_nc():
    nc = bacc.Bacc()

    ef = nc.dram_tensor("ef", [RADIAL, E_CORE], FP32, kind="ExternalInput")
    x4 = nc.dram_tensor("x4", [128, E_CORE], BF16_DT, kind="ExternalInput")
    at = nc.dram_tensor("at", [SUB, N_ST * NSH], BF16_DT,
                        kind="ExternalInput")
    rl = nc.dram_tensor("rl", [SUB, N_ST], FP32, kind="ExternalInput")
    w1 = nc.dram_tensor("w1", [RADIAL, HID], FP32, kind="ExternalInput")
    w2 = nc.dram_tensor("w2", [HID, HID], FP32, kind="ExternalInput")
    w3 = nc.dram_tensor("w3", [HID, HID], FP32, kind="ExternalInput")
    wg = nc.dram_tensor("wg", [128, N_CHUNK * LO], BF16_DT, kind="ExternalInput")
    iota = nc.dram_tensor("iota", [128, 128], FP32, kind="ExternalInput")
    out = nc.dram_tensor("out", [NODES_PER_CORE, F_OUT], FP32, kind="ExternalOutput")

    with tile.TileContext(nc) as tc, ExitStack() as ctx:
        const_p = ctx.enter_context(tc.tile_pool(name="const", bufs=1))
        stream_p = ctx.enter_context(tc.tile_pool(name="stream", bufs=3))
        win_p = ctx.enter_context(tc.tile_pool(name="win", bufs=2))
        chunk_p = ctx.enter_context(tc.tile_pool(name="chunk", bufs=3))
        psum_mlp = ctx.enter_context(tc.tile_pool(name="pmlp", bufs=2, space="PSUM"))
        psum_u = ctx.enter_context(tc.tile_pool(name="pu", bufs=3, space="PSUM"))
        psum_ut = ctx.enter_context(tc.tile_pool(name="put", bufs=2, space="PSUM"))
        psum_acc = ctx.enter_context(tc.tile_pool(name="pacc", bufs=1, space="PSUM"))
        dram_p = ctx.enter_context(tc.tile_pool(name="dram", bufs=3, space="DRAM"))
        tail_p = ctx.enter_context(tc.tile_pool(name="tail", bufs=1))

        # ---- one-time constants into SBUF ----
        w1_sb = const_p.tile([RADIAL, HID], FP32)
        nc.scalar.dma_start(w1_sb[:], w1[:])
        w2_sb = const_p.tile([HID, HID], FP32)
        nc.scalar.dma_start(w2_sb[:], w2[:])
        w3_sb = const_p.tile([HID, HID], FP32)
        nc.scalar.dma_start(w3_sb[:], w3[:])
        wg_sb = const_p.tile([128, N_CHUNK * LO], BF16_DT)
        nc.scalar.dma_start(wg_sb[:], wg[:])
        iota_sb = const_p.tile([128, 128], FP32)
        nc.scalar.dma_start(iota_sb[:], iota[:])
        ident_sb = const_p.tile([128, 128], FP32)
        make_identity(nc, ident_sb[:])

        for w in range(WINS_PER_CORE):
            # window-level streams
            at_sb = win_p.tile([SUB, SUBS_PER_WIN * NSH], BF16_DT, tag="at")
            nc.scalar.dma_start(
                at_sb[:],
                at[:, w * SUBS_PER_WIN * NSH:(w + 1) * SUBS_PER_WIN * NSH])
            rl_sb = win_p.tile([SUB, SUBS_PER_WIN], FP32, tag="rl")
            nc.scalar.dma_start(
                rl_sb[:], rl[:, w * SUBS_PER_WIN:(w + 1) * SUBS_PER_WIN])
            ut_sb = win_p.tile([SUB, SUBS_PER_WIN, LO], BF16_DT, tag="ut")
            msgs_sb = win_p.tile([SUB, SUBS_PER_WIN, F_OUT], BF16_DT, tag="msgs")

            # S matrices for the whole window in one DVE op
            s_all = win_p.tile([SUB, SUBS_PER_WIN, WIN], BF16_DT, tag="s")
            nc.vector.tensor_tensor(
                out=s_all[:],
                in0=iota_sb[:, None, :].to_broadcast([SUB, SUBS_PER_WIN, WIN]),
                in1=rl_sb[:, :, None].to_broadcast([SUB, SUBS_PER_WIN, WIN]),
                op=mybir.AluOpType.is_equal)

            e_off = 0  # edge offset within window
            for tsz in TILE_SIZES:
                base = w * WIN_E + e_off          # global edge-slot offset
                nsub = tsz // SUB

                ef_sb = stream_p.tile([RADIAL, 512], FP32, tag="ef")
                nc.scalar.dma_start(ef_sb[:, :tsz], ef[:, base:base + tsz])

                # Xrep[p, e] = node_feats[senders[e], p // 4] (host-gathered)
                x_sb = stream_p.tile([128, 512], BF16_DT, tag="x")
                nc.scalar.dma_start(x_sb[:, :tsz], x4[:, base:base + tsz])

                # --- MLP (feature-major) ---
                z1 = psum_mlp.tile([HID, 512], FP32, tag="z")
                nc.tensor.matmul(out=z1[:, :tsz], lhsT=w1_sb[:], rhs=ef_sb[:, :tsz],
                                 start=True, stop=True, skip_group_check=True)
                h1 = stream_p.tile([HID, 512], FP32, tag="h1")
                nc.scalar.activation(h1[:, :tsz], z1[:, :tsz],
                                     ACT_FUNC)
                z2 = psum_mlp.tile([HID, 512], FP32, tag="z")
                nc.tensor.matmul(out=z2[:, :tsz], lhsT=w2_sb[:], rhs=h1[:, :tsz],
                                 start=True, stop=True, skip_group_check=True)
                h2 = stream_p.tile([HID, 512], FP32, tag="h2")
                nc.scalar.activation(h2[:, :tsz], z2[:, :tsz],
                                     ACT_FUNC)
                # last layer with W3 columns permuted: row k' = 16j + c is
                # MLP feature k = 4c + j
                z3 = psum_mlp.tile([HID, 512], FP32, tag="z")
                nc.tensor.matmul(out=z3[:, :tsz], lhsT=w3_sb[:], rhs=h2[:, :tsz],
                                 start=True, stop=True, skip_group_check=True)
                h3p = stream_p.tile([HID, 512], BF16_DT, tag="h3")
                nc.scalar.activation(h3p[:, :tsz], z3[:, :tsz],
                                     ACT_FUNC)

                # --- broadcast h3 along partitions (32x) via DRAM bounce ---
                # h3p rows are already (j, c)-ordered, so the bounce write is
                # a plain copy; reads are split across sync + gpsimd queues.
                # h3bc[p=(j,i), c, e] = h3p[16j + c, e]
                # The DGE splits a DMA across SDMA engines by the OUTERMOST
                # AP dim, so put the 32-way replication dim outermost to
                # engage all 16 engines (outer dim 4 -> only 4 engines).
                # Partition convention p = 4i + j: the dest partition dim
                # pairs positionally with src dims (32, 4, c*e), making the
                # 32-way broadcast the OUTER split factor so the DGE spreads
                # descriptors over 16 SDMA engines (outer dim 4 -> only 4).
                pool = chunk_p if tsz == 512 else tail_p
                h3d = dram_p.tile([HID, tsz], BF16_DT, tag=f"h3d{tsz}")
                nc.sync.dma_start(h3d[:], h3p[:, :tsz])
                h3bc = pool.tile([128, N_CHUNK, tsz], BF16_DT, tag=f"h3bc{tsz}")
                dst = h3bc[:].rearrange("p c e -> p (c e)")
                src = h3d[:].rearrange("(j c) e -> j (c e)", j=4)
                src = src[None, :, :].to_broadcast([32, 4, N_CHUNK * tsz])
                nc.sync.dma_start(dst, src)

                # --- outer product (batched) + einsum chunks ---
                a_all = pool.tile([128, N_CHUNK, tsz], BF16_DT, tag=f"a{tsz}")
                for g in range(2):
                    nc.vector.tensor_tensor(
                        out=a_all[:, 8 * g:8 * g + 8, :tsz],
                        in0=h3bc[:, 8 * g:8 * g + 8, :tsz],
                        in1=x_sb[:, None, :tsz].to_broadcast([128, 8, tsz]),
                        op=mybir.AluOpType.mult)
                u_ps = psum_u.tile([LO, 512], FP32, tag="u")
                for c in range(N_CHUNK):
                    nc.tensor.matmul(out=u_ps[:, :tsz],
                                     lhsT=wg_sb[:, c * LO:(c + 1) * LO],
                                     rhs=a_all[:, c, :tsz],
                                     start=(c == 0), stop=(c == N_CHUNK - 1),
                                     skip_group_check=True)

                # --- transpose u to edge-major ---
                u_sb = stream_p.tile([LO, 512], FP32, tag="usb")
                nc.scalar.copy(u_sb[:, :tsz], u_ps[:, :tsz])
                ut_ps = psum_ut.tile([128, 4, LO], FP32, tag="utp")
                for s in range(nsub):
                    nc.tensor.transpose(
                        out=ut_ps[:, s, :],
                        in_=u_sb[:, s * SUB:(s + 1) * SUB],
                        identity=ident_sb[:LO, :LO])
                st0 = e_off // SUB
                nc.scalar.copy(ut_sb[:, st0:st0 + nsub, :], ut_ps[:, :nsub, :])

                e_off += tsz

            # --- msgs = ut * attr (attr broadcast along c; DVE 1x) ---
            lofs = (0, 1, 4)
            for l in range(NL):
                dim = L_DIMS[l]
                u_ap = ut_sb[:, :, None, l * C:(l + 1) * C].to_broadcast(
                    [SUB, SUBS_PER_WIN, dim, C])
                a_ap = at_sb[:].rearrange("p (s m) -> p s m", m=NSH)[
                    :, :, lofs[l]:lofs[l] + dim]
                a_ap = a_ap[:, :, :, None].to_broadcast(
                    [SUB, SUBS_PER_WIN, dim, C])
                nc.vector.tensor_tensor(
                    out=msgs_sb[:, :, lofs[l] * C:(lofs[l] + dim) * C].rearrange(
                        "p s (m c) -> p s m c", c=C),
                    in0=u_ap, in1=a_ap, op=mybir.AluOpType.mult)

            # --- scatter: psum_out += S.T @ msgs per subtile ---
            acc = psum_acc.tile([WIN, F_OUT], FP32, tag="acc")
            for st in range(SUBS_PER_WIN):
                nc.tensor.matmul(out=acc[:], lhsT=s_all[:, st, :],
                                 rhs=msgs_sb[:, st, :],
                                 start=(st == 0), stop=(st == SUBS_PER_WIN - 1),
                                 skip_group_check=True)

            out_sb = stream_p.tile([WIN, F_OUT], FP32, tag="osb")
            nc.scalar.copy(out_sb[:], acc[:])
            nc.scalar.dma_start(out[w * WIN:(w + 1) * WIN, :], out_sb[:])

    nc.compile()
    return nc


def _host_prep(node_feats, edge_attrs, edge_feats, senders, receivers,
               W1, W2, W3, Wgen):
    """Sort/shard edges by receiver window, build per-core input maps."""
    senders = np.asarray(senders).astype(np.int64)
    receivers = np.asarray(receivers).astype(np.int64)
    node_feats = np.asarray(node_feats, dtype=np.float32)
    edge_attrs = np.asarray(edge_attrs, dtype=np.float32)
    edge_feats = np.asarray(edge_feats, dtype=np.float32)

    n_win_total = N_CORES * WINS_PER_CORE  # 80
    win_id = receivers // WIN
    order = np.argsort(win_id, kind="stable")
    counts = np.bincount(win_id, minlength=n_win_total)
    assert counts.max() <= WIN_E, f"window overflow: {counts.max()} > {WIN_E}"
    starts = np.zeros(n_win_total + 1, np.int64)
    np.cumsum(counts, out=starts[1:])

    # slot arrays (padded); padding edges: ef=0, attr=0 -> msgs contribution 0
    E_TOT = N_CORES * E_CORE
    ef_s = np.zeros((E_TOT, RADIAL), np.float32)
    at_s = np.zeros((E_TOT, NSH), np.float32)
    rl_s = np.zeros(E_TOT, np.float32)
    sd_s = np.zeros(E_TOT, np.int64)

    slot_base = np.arange(n_win_total) * WIN_E
    # positions for real edges
    within = np.arange(len(order)) - starts[win_id[order]]
    slots = slot_base[win_id[order]] + within
    ef_s[slots] = edge_feats[order]
    at_s[slots] = edge_attrs[order] * np.float32(1.0 / np.sqrt(AVG_NUM_NEIGHBORS))
    rl_s[slots] = (receivers[order] % WIN).astype(np.float32)
    sd_s[slots] = senders[order]

    # host-side sender gather, replicated 4x along partitions (bf16)
    xg = node_feats[sd_s].astype(BF16)            # [E_TOT, 32]

    # weights with fan-in scales folded
    w1 = (W1 * (1.0 / np.sqrt(RADIAL))).astype(np.float32)
    w2 = (W2 * (1.0 / np.sqrt(HID))).astype(np.float32)
    w3 = (W3 * (1.0 / np.sqrt(HID))).astype(np.float32)
    # permute W3 columns so output row k' = 16j + c holds feature k = 4c + j
    perm = np.empty(HID, np.int64)
    for j in range(4):
        for c in range(N_CHUNK):
            perm[16 * j + c] = 4 * c + j
    w3p = np.ascontiguousarray(w3[:, perm])
    # partition p = 4i + j: wg[c][p, lo] = Wgen[4c + p%4, l, o, p//4] / sqrt(HID*C)
    wgen = np.asarray(Wgen, dtype=np.float32) * np.float32(1.0 / np.sqrt(HID * C))
    p = np.arange(128)
    wgc = np.zeros((N_CHUNK, 128, NL, C), np.float32)
    for c in range(N_CHUNK):
        wgc[c] = wgen[4 * c + p % 4][p, :, :, p // 4].reshape(128, NL, C)
    # -> [128, 16*96]: chunk-major along free dim
    wgc = wgc.reshape(N_CHUNK, 128, LO).transpose(1, 0, 2).reshape(
        128, N_CHUNK * LO)
    wgc = wgc.astype(BF16)

    iota = np.broadcast_to(np.arange(128, dtype=np.float32),
                           (128, 128)).copy()

    in_maps = []
    for m in range(N_CORES):
        sl = slice(m * E_CORE, (m + 1) * E_CORE)
        ef_c = ef_s[sl]      # [E_CORE, 8]
        at_c = at_s[sl]      # [E_CORE, 9]
        rl_c = rl_s[sl]
        # attrs compact bf16, subtile-major (expanded on device by gpsimd)
        atc = np.ascontiguousarray(
            at_c.astype(BF16).reshape(N_ST, SUB, NSH).transpose(1, 0, 2).reshape(
                SUB, N_ST * NSH))
        x4_c = np.ascontiguousarray(
            np.repeat(xg[sl].T, 4, axis=0))                   # [128, E_CORE]
        in_maps.append({
            "ef": np.ascontiguousarray(ef_c.T),
            "at": atc,
            "rl": np.ascontiguousarray(
                rl_c.reshape(N_ST, SUB).T),
            "x4": x4_c,
            "w1": w1, "w2": w2, "w3": w3p, "wg": wgc,
            "iota": iota,
        })
    return in_maps


def kernel(node_feats, edge_attrs, edge_feats, senders, receivers,
           W1, W2, W3, Wgen):
    in_maps = _host_prep(node_feats, edge_attrs, edge_feats, senders, receivers,
                         W1, W2, W3, Wgen)
    if "nc" not in _CACHED:
        _CACHED["nc"] = _build_nc()
    nc = _CACHED["nc"]
    res = run_bass_kernel_spmd(nc, in_maps, core_ids=list(range(N_CORES)))
    outs = [res.results[m]["out"] for m in range(N_CORES)]
    full = np.concatenate(outs, axis=0)[:N_NODES]          # [10000, 288]
    out = full.reshape(N_NODES, NSH, C).transpose(0, 2, 1)  # [10000, 32, 9]
    return np.ascontiguousarray(out.astype(np.float32))
